# revision 27
# baseline (speedup 1.0000x reference)
"""Trainium2 Bass kernel for the batched 2D Kalman filter (nn_KalmanFilterWrapper).

Math
----
The reference runs, per trajectory, a Kalman filter over T=4096 steps with a
constant-velocity model.  The gain/covariance recursion (Riccati) is
data-independent, so the scan collapses to a linear time-varying recurrence

    x_t = A_t x_{t-1} + k_t z_t,        y_t = x_t[0]

with coefficients shared across the whole batch.  The 4-state filter decouples
into two identical 2-state (position, velocity) scalar filters — one per
coordinate — giving B*2 = 8192 independent scalar sequences.

The recurrence coefficients converge to steady state by t~135, and the steady
transition matrix has spectral radius 0.9315, so the filter's impulse response
g_d decays below 1e-6 by d=192.  Each aligned 128-step output chunk therefore
depends (to ~1e-5, vs a 2e-2 accuracy gate) only on the 256 measurements in
its own and the preceding 128-step input block:

    y[128*ci : 128*(ci+1)] = W_lo @ z_prev_block + W_hi @ z_this_block

where (W_lo, W_hi) are one shared Toeplitz pair built from g for all ci >= 2,
exact time-varying matrices for ci == 1, and a single exact lower-triangular
matrix for ci == 0 (which also folds in the x0 = [z_0, 0] initial condition).
All 32 chunks are INDEPENDENT matmuls — no serial carry chain at all.

Data movement (HBM floor ~358 GB/s per core):
  - inputs: fp8e4m3 with first-order NOISE-SHAPED quantization (error
    feedback e_t = v_t - fp8(v_t), v_t = z_t + 0.909 e_{t-1}) — 4 MB/core.
    The shaping pushes the fp8 rounding noise to high frequencies where the
    Kalman filter's transfer function is small: the in-band error is
    ||g*h||/||g|| = 0.42 of the raw 2.6e-2 fp8 error.  The fp8 tensor feeds
    the PE directly as the MOVING operand against bf16 stationary weights
    (mixed-dtype matmul, verified bit-exact on TRN2) — zero upconvert cost.
  - outputs: int8 round(y * s), saturating — 4 MB/core; s is folded into the
    weight slots (48 for the transient blocks 0-1 whose |y| reaches 4.0,
    104 for steady blocks where sigma(y)=0.32), and the host divides it back
    out after the gather.  PSUM evictions (fp32 -> int8, round-to-nearest)
    alternate between ACT and DVE at full copy speed.
  - matmuls accumulate in fp32 PSUM.
Host-sim l2 relative error: 1.58e-2 (gate 2e-2, deterministic for the fixed
harness input); output int8 quantization ~0.95e-2 and shaped-fp8 input
~1.16e-2 dominate; bf16 weights + 256-tap truncation are ~0.2e-2.

Sharding: data-parallel across 8 NeuronCores, 512 trajectories (1024 scalar
sequences) per core.  Device layout is [128 partitions = t%128, block-major
free dim], pre-swizzled on the host so every DMA is contiguous; small leading
input units and trailing solo output stores keep pipeline ramp/drain short.
"""

import numpy as np
import ml_dtypes

import concourse.bass as bass
import concourse.bacc as bacc
import concourse.mybir as mybir
from concourse.bass_utils import run_bass_kernel_spmd
from concourse.tile import TileContext

# Problem constants (hardcoded per harness contract).
B = 4096
T = 4096
DT = 1.0
PROCESS_VARIANCE = 1e-05
MEASUREMENT_VARIANCE = 0.1
INIT_ERROR = 1.0

N_CORES = 8
NCOLS = (B * 2) // N_CORES  # 1024 scalar sequences per core
CHUNK = 512                 # matmul free dim (one fp32 PSUM bank)
NBLK = T // 128             # 32 output chunks
NSLOT = 5                   # weight matrices: W0, Wlo1, Whi1, WloS, WhiS

BF16 = mybir.dt.bfloat16
F8 = mybir.dt.float8e4
I8 = mybir.dt.int8
F32 = mybir.dt.float32
NPBF16 = ml_dtypes.bfloat16
NPF8 = ml_dtypes.float8_e4m3

# Output quantization scales, folded per weight slot; the host divides them
# back out after the gather.  Transient blocks 0-1 see |y| up to 4.0 (the
# filter starts at gain ~1), steady blocks are ~N(0, 0.32^2).
OSCALE_T = 48.0   # blocks 0, 1 (slots 0-2)
OSCALE_S = 104.0  # blocks 2..31 (slots 3-4)
# First-order noise-shaping feedback coefficient (min ||g*h|| monic h).
SHAPE_FB = 0.9091


def _precompute_lhsT():
    """Host-side Riccati + chunk weight matrices, float64 -> bf16.

    Returns [128, 5*128] bf16; slot s holds lhsT = W_s.T so that
    matmul(out, lhsT, z) computes out[t, n] = sum_k W_s[t, k] z[k, n].
    """
    F = np.array([[1.0, DT], [0.0, 1.0]], dtype=np.float64)
    I2 = np.eye(2, dtype=np.float64)
    P = INIT_ERROR * I2.copy()
    A = np.zeros((T, 2, 2), dtype=np.float64)
    k = np.zeros((T, 2), dtype=np.float64)
    for t in range(T):
        Pp = F @ P @ F.T + PROCESS_VARIANCE * I2
        s = Pp[0, 0] + MEASUREMENT_VARIANCE
        kt = Pp[:, 0] / s
        k[t] = kt
        KH = np.zeros((2, 2), dtype=np.float64)
        KH[:, 0] = kt
        P = (I2 - KH) @ Pp
        A[t] = (I2 - KH) @ F

    # Exact input->output operator over the first 256 steps.  Rc[:, j] is the
    # coefficient of measurement z_j in the current state; the initial state
    # is x_{-1} = [z_0, 0].
    W = np.zeros((256, 256), dtype=np.float64)
    Rc = np.zeros((2, 256), dtype=np.float64)
    Rc[0, 0] = 1.0
    for t in range(256):
        Rc = A[t] @ Rc
        Rc[:, t] += k[t]
        W[t] = Rc[0]

    # Steady-state impulse response g_d = [Ainf^d kinf][0].
    g = np.zeros(256, dtype=np.float64)
    vv = k[-1].copy()
    for d in range(256):
        g[d] = vv[0]
        vv = A[-1] @ vv
    m, kk = np.mgrid[0:128, 0:128]
    WloS = g[m + 128 - kk]
    WhiS = np.where(m >= kk, g[np.abs(m - kk)], 0.0)

    slots = [
        (W[0:128, 0:128], OSCALE_T),      # chunk 0 (exact, incl. init cond)
        (W[128:256, 0:128], OSCALE_T),    # chunk 1 lo (exact transient)
        (W[128:256, 128:256], OSCALE_T),  # chunk 1 hi
        (WloS, OSCALE_S),                 # chunks 2..31 lo (steady Toeplitz)
        (WhiS, OSCALE_S),                 # chunks 2..31 hi
    ]
    lhsT = np.zeros((128, NSLOT * 128), dtype=np.float64)
    for s, (Ws, sc) in enumerate(slots):
        lhsT[:, s * 128:(s + 1) * 128] = Ws.T * sc
    return np.ascontiguousarray(lhsT.astype(NPBF16))


def _build_nc():
    # Device layout (host pre-swizzled): z/v are [128 partitions, 32 blocks x
    # 1024 seqs]; partition p of block bi holds time step 128*bi + p.  Every
    # DMA is then fully contiguous per partition line.
    nc = bacc.Bacc()
    z = nc.dram_tensor("z", [128, NBLK * NCOLS], F8, kind="ExternalInput")
    u = nc.dram_tensor("u", [128, NSLOT * 128], BF16, kind="ExternalInput")
    v = nc.dram_tensor("v", [128, NBLK * NCOLS], I8, kind="ExternalOutput")

    nchunks = NCOLS // CHUNK

    # DMA granularity: a dma_start's descriptor generation (DIRECT2D)
    # occupies its issuing HWDGE ring ~0.6-0.7us, and the two rings (sync /
    # scalar) generate in parallel.  fp8 blocks are 128 KiB; small leading
    # units shorten pipeline ramp, then 512 KiB groups of 4, alternating
    # rings so consecutive units' descriptor gens overlap.
    GROUP = 4
    # Input units: solos, then pairs through block 19 (finer completion
    # granularity where the stream runs just-in-time against the PE), then
    # quads.  The PE consumes a block per ~0.86us; input completions land
    # every ~0.7us per pair, so group-boundary stalls stay under the noise.
    in_units = [[0], [1]]
    bi = 2
    while bi < 20:
        in_units.append([bi, bi + 1])
        bi += 2
    while bi < NBLK:
        in_units.append(list(range(bi, bi + GROUP)))
        bi += GROUP
    # Output units: quads through block 23, then pairs, then solos — the
    # tail stores shrink so the post-stream drain is one small transfer.
    out_units = []
    ci = 0
    while ci < 24:
        out_units.append(list(range(ci, ci + GROUP)))
        ci += GROUP
    out_units += [[24, 25], [26, 27], [28], [29], [30], [31]]
    SOLO_OUT = 4
    out_unit_of = {}
    for unit in out_units:
        for c in unit:
            out_unit_of[c] = unit

    with TileContext(nc) as tc:
        with (
            tc.tile_pool(name="consts", bufs=1) as cpool,
            tc.tile_pool(name="zsolo", bufs=3) as zsolo,
            tc.tile_pool(name="zgrp", bufs=8) as zgrp,
            tc.tile_pool(name="vsolo", bufs=SOLO_OUT) as vsolo,
            tc.tile_pool(name="vgrp", bufs=6) as vgrp,
            tc.tile_pool(name="psum", bufs=8, space="PSUM") as ppool,
        ):
            # The SDMA engines starve the scalar ring's queue while the sync
            # ring has descriptors pending (observed ~2.5-3us stalls whenever
            # a block-0/1-critical transfer rode scalar).  So ALL inputs ride
            # the sync ring in consumption order; stores ride scalar.
            # Slot-0 weights first (they gate every matmul), then z block 0,
            # then the rest of the weights (needed from block 1 on).
            u_tile = cpool.tile([128, NSLOT * 128], BF16)
            # one-line dummy read rings the sync queue's doorbell first: the
            # first-use queue activation (~1us: engine wake + ring init) runs
            # concurrently with the real descriptor gens instead of after
            # them.  Its single descriptor costs ~50ns of engine time.
            dummy = cpool.tile([1, 128], BF16, name="qwarm")
            nc.sync.dma_start(dummy[0:1, :], u[0:1, bass.ds(0, 128)])
            nc.sync.dma_start(u_tile[:, 0:128], u[:, bass.ds(0, 128)])

            # PE HAM warm-up: the clock gate starts at 1.2 GHz and needs
            # ~3.4us of sustained activity to reach 2.4 GHz.  Burn scratch
            # matmuls (results never read) while the first input DMAs are in
            # flight.  The zero tile is memset on the otherwise-idle GpSimd
            # and serves as BOTH operands (fp8 stationary x fp8 moving), so
            # the warm-ups wait on nothing else — in particular not on the
            # weights DMA.  Two cold (1.2 GHz) warm-ups fill the ~0.9us gap
            # until z block 0 + slot-0 weights land.
            warm = zsolo.tile([128, 128 + CHUNK], F8, name="warm", tag="zp")
            nc.gpsimd.memset(warm[:, :], 0)
            for wi in range(2):
                wps = ppool.tile([128, CHUNK], F32, name=f"wps{wi}", tag="ps")
                nc.tensor.matmul(
                    wps[:, :],
                    warm[:, bass.ds(0, 128)],
                    warm[:, bass.ds(128, CHUNK)],
                    start=True,
                    stop=True,
                )

            # Input loads: fp8, units alternating between the two HWDGE
            # rings. zloc[bi]=(tile,col0).
            zloc = {}
            for ui, unit in enumerate(in_units):
                w = len(unit) * NCOLS
                pool = zsolo if len(unit) == 1 else zgrp
                zp = pool.tile([128, w], F8, name=f"z{unit[0]}", tag="zp")
                ring = nc.sync
                if unit == [0]:
                    # block 0 in two half-DMAs: the first matmul only needs
                    # cols 0:512, and tile deps are range-tracked, so the
                    # pipeline starts half a transfer earlier; the remaining
                    # weight slots follow, ahead of z block 1, matching
                    # consumption order
                    ring.dma_start(zp[:, 0:CHUNK], z[:, bass.ds(0, CHUNK)])
                    ring.dma_start(
                        zp[:, CHUNK:NCOLS], z[:, bass.ds(CHUNK, NCOLS - CHUNK)]
                    )
                    nc.sync.dma_start(
                        u_tile[:, 128:NSLOT * 128],
                        u[:, bass.ds(128, (NSLOT - 1) * 128)],
                    )
                else:
                    ring.dma_start(
                        zp[:, :], z[:, bass.ds(unit[0] * NCOLS, w)]
                    )
                for si, b in enumerate(unit):
                    zloc[b] = (zp, si * NCOLS)

            vloc = {}
            evict = 0
            for ci in range(NBLK):
                unit = out_unit_of[ci]
                if ci == unit[0]:
                    w = len(unit) * NCOLS
                    pool = vsolo if len(unit) == 1 else vgrp
                    vt = pool.tile([128, w], I8, name=f"v{ci}", tag="vout")
                    for si, c in enumerate(unit):
                        vloc[c] = (vt, si * NCOLS)
                vout, vcol0 = vloc[ci]
                zhi, hcol0 = zloc[ci]
                ps = [
                    ppool.tile([128, CHUNK], F32, name=f"ps{cc}", tag="ps")
                    for cc in range(nchunks)
                ]
                if ci == 0:
                    for cc in range(nchunks):
                        nc.tensor.matmul(
                            ps[cc][:, :],
                            u_tile[:, bass.ds(0, 128)],
                            zhi[:, bass.ds(hcol0 + cc * CHUNK, CHUNK)],
                            start=True,
                            stop=True,
                        )
                else:
                    zlo, lcol0 = zloc[ci - 1]
                    lo_slot, hi_slot = (1, 2) if ci == 1 else (3, 4)
                    # lo over both col-chunks, then hi: consecutive matmuls
                    # share the stationary operand.
                    for cc in range(nchunks):
                        nc.tensor.matmul(
                            ps[cc][:, :],
                            u_tile[:, bass.ds(lo_slot * 128, 128)],
                            zlo[:, bass.ds(lcol0 + cc * CHUNK, CHUNK)],
                            start=True,
                            stop=False,
                        )
                    for cc in range(nchunks):
                        nc.tensor.matmul(
                            ps[cc][:, :],
                            u_tile[:, bass.ds(hi_slot * 128, 128)],
                            zhi[:, bass.ds(hcol0 + cc * CHUNK, CHUNK)],
                            start=False,
                            stop=True,
                        )
                # split PSUM evictions across DVE and ACT, strictly
                # alternating so neither engine queues a long run
                if ci == NBLK - 1:
                    # final block: chunk 0 (psum ready one matmul early) goes
                    # whole on DVE; chunk 1 — the last psum — splits across
                    # BOTH engines so the final eviction finishes ~0.35us
                    # after the last matmul
                    nc.vector.tensor_copy(
                        vout[:, bass.ds(vcol0, CHUNK)], ps[0][:, :]
                    )
                    half = CHUNK // 2
                    nc.scalar.copy(
                        vout[:, bass.ds(vcol0 + CHUNK, half)], ps[1][:, 0:half]
                    )
                    nc.vector.tensor_copy(
                        vout[:, bass.ds(vcol0 + CHUNK + half, half)],
                        ps[1][:, half:CHUNK],
                    )
                else:
                    for cc in range(nchunks):
                        cols = bass.ds(vcol0 + cc * CHUNK, CHUNK)
                        if evict % 2 == 0:
                            nc.vector.tensor_copy(vout[:, cols], ps[cc][:, :])
                        else:
                            nc.scalar.copy(vout[:, cols], ps[cc][:, :])
                        evict += 1
                # Output stores alternate between the two HWDGE rings (sync /
                # scalar) so consecutive stores overlap with the input loads
                # that lead the sync ring.
                if ci == unit[-1]:
                    w = len(unit) * NCOLS
                    vt0, _ = vloc[unit[0]]
                    if ci == NBLK - 1:
                        # final block: halves on both rings so the tail
                        # descriptor gens and transfers run in parallel
                        half = w // 2
                        nc.sync.dma_start(
                            v[:, bass.ds(unit[0] * NCOLS, half)], vt0[:, 0:half]
                        )
                        nc.scalar.dma_start(
                            v[:, bass.ds(unit[0] * NCOLS + half, half)],
                            vt0[:, half:w],
                        )
                    elif unit[0] >= 24:
                        # trailing pairs/solos alternate rings — by now the
                        # sync ring's input queue has drained, so both get
                        # service
                        ring = nc.sync if unit[0] in (24, 28, 30) else nc.scalar
                        ring.dma_start(
                            v[:, bass.ds(unit[0] * NCOLS, w)], vt0[:, :]
                        )
                    else:
                        # bulk stores ride the (starvable) scalar ring; their
                        # only deadline is SBUF buffer recycling, which has
                        # ~24 blocks of slack
                        nc.scalar.dma_start(
                            v[:, bass.ds(unit[0] * NCOLS, w)], vt0[:, :]
                        )
    nc.finalize()  # Bacc.compile(): splits multi-waits, allocates registers
    return nc


_CACHE = {}


def _encode_fp8_shaped(zt: np.ndarray) -> np.ndarray:
    """First-order noise-shaped fp8e4m3 quantization along the time axis.

    zt: [T, ncols] float32.  Returns [T, ncols] fp8 codes whose rounding
    error is spectrally shaped by H(z) = 1 - SHAPE_FB * z^-1.
    """
    out = np.empty(zt.shape, dtype=NPF8)
    e = np.zeros(zt.shape[1], dtype=np.float32)
    for t in range(T):
        vrow = zt[t] + SHAPE_FB * e
        qrow = vrow.astype(NPF8)
        out[t] = qrow
        e = vrow - qrow.astype(np.float32)
    return out


def _run(x_seq: np.ndarray, trace: bool = False):
    if "nc" not in _CACHE:
        _CACHE["nc"] = _build_nc()
        _CACHE["u"] = _precompute_lhsT()
    nc = _CACHE["nc"]
    u_all = _CACHE["u"]

    x = np.asarray(x_seq, dtype=np.float32)
    assert x.shape == (B, T, 2), x.shape

    # [B, T, 2] -> [T, B*2]; column n = 2*b + c.  Noise-shaped fp8 encode,
    # then each core's [T, NCOLS] shard is swizzled into the device layout
    # [128, NBLK*NCOLS] (partition = t % 128, block-major free dim) so DMAs
    # are contiguous.
    zt = np.ascontiguousarray(x.transpose(1, 0, 2).reshape(T, B * 2))
    z8 = _encode_fp8_shaped(zt)

    def swizzle(a, ncols_lo, ncols_hi):
        nb = a.shape[0] // 128
        return np.ascontiguousarray(
            a[:, ncols_lo:ncols_hi]
            .reshape(nb, 128, ncols_hi - ncols_lo)
            .transpose(1, 0, 2)
            .reshape(128, nb * (ncols_hi - ncols_lo))
        )

    in_maps = [
        {"z": swizzle(z8, i * NCOLS, (i + 1) * NCOLS), "u": u_all}
        for i in range(N_CORES)
    ]
    res = run_bass_kernel_spmd(nc, in_maps, core_ids=list(range(N_CORES)), trace=trace)

    # inverse swizzle: [128, NBLK*NCOLS] -> [T, NCOLS], concat cores, dequant
    vt = np.concatenate(
        [
            r["v"].reshape(128, NBLK, NCOLS).transpose(1, 0, 2).reshape(T, NCOLS)
            for r in res.results
        ],
        axis=1,
    )  # [T, B*2] int8 = round(y * scale)
    vf = vt.astype(np.float32)
    vf[:256] /= OSCALE_T
    vf[256:] /= OSCALE_S
    out = np.ascontiguousarray(vf.reshape(T, B, 2).transpose(1, 0, 2))
    return out, res


def kernel(x_seq: np.ndarray) -> np.ndarray:
    out, _ = _run(x_seq, trace=False)
    return out


# revision 29
# speedup vs baseline: 1.0219x; 1.0219x over previous
"""Trainium2 Bass kernel for the batched 2D Kalman filter (nn_KalmanFilterWrapper).

Math
----
The reference runs, per trajectory, a Kalman filter over T=4096 steps with a
constant-velocity model.  The gain/covariance recursion (Riccati) is
data-independent, so the scan collapses to a linear time-varying recurrence

    x_t = A_t x_{t-1} + k_t z_t,        y_t = x_t[0]

with coefficients shared across the whole batch.  The 4-state filter decouples
into two identical 2-state (position, velocity) scalar filters — one per
coordinate — giving B*2 = 8192 independent scalar sequences.

The recurrence coefficients converge to steady state by t~135, and the steady
transition matrix has spectral radius 0.9315, so the filter's impulse response
g_d decays below 1e-6 by d=192.  Each aligned 128-step output chunk therefore
depends (to ~1e-5, vs a 2e-2 accuracy gate) only on the 256 measurements in
its own and the preceding 128-step input block:

    y[128*ci : 128*(ci+1)] = W_lo @ z_prev_block + W_hi @ z_this_block

where (W_lo, W_hi) are one shared Toeplitz pair built from g for all ci >= 2,
exact time-varying matrices for ci == 1, and a single exact lower-triangular
matrix for ci == 0 (which also folds in the x0 = [z_0, 0] initial condition).
All 32 chunks are INDEPENDENT matmuls — no serial carry chain at all.

Data movement (HBM floor ~358 GB/s per core; the kernel is PE-bound at
~27us of matmul streaming, with DMA at ~25us fully hidden under it):
  - inputs: fp8e4m3 with first-order NOISE-SHAPED quantization (error
    feedback e_t = v_t - fp8(v_t), v_t = z_t + 0.909 e_{t-1}) — 4 MB/core.
    The shaping pushes the fp8 rounding noise to high frequencies where the
    Kalman filter's transfer function is small: the in-band error is
    ||g*h||/||g|| = 0.42 of the raw 2.6e-2 fp8 error.  The fp8 tensor feeds
    the PE directly as the MOVING operand against bf16 stationary weights
    (mixed-dtype matmul, verified bit-exact on TRN2) — zero upconvert cost.
    ALL input transfers ride the sync HWDGE ring in exact consumption order
    (slot-0 weights, z block 0, remaining weights, z block 1, pairs, quads):
    the SDMA engines leave a freshly-doorbelled second queue unserviced for
    ~2-3us while the first has pending descriptors, so any early-needed
    transfer on the scalar ring stalls the PE (measured, repeatedly).
  - outputs: int8 round(y * s), saturating — 4 MB/core; s is folded into the
    weight slots (48 for the transient blocks 0-1 whose |y| reaches 4.0,
    104 for steady blocks where sigma(y)=0.32), and the host divides it back
    out after the gather.  PSUM evictions (fp32 -> int8, round-to-nearest)
    alternate between ACT and DVE at full copy speed.  Bulk stores ride the
    (starvable) scalar ring; the trailing pairs/solos alternate rings and
    the final block is split across both so the drain is short.
  - matmuls accumulate in fp32 PSUM.
Host-sim l2 relative error: 1.58e-2 (gate 2e-2, deterministic for the fixed
harness input); output int8 quantization ~0.95e-2 and shaped-fp8 input
~1.16e-2 dominate; bf16 weights + 256-tap truncation are ~0.2e-2.

Sharding: data-parallel across 8 NeuronCores, 512 trajectories (1024 scalar
sequences) per core.  Device layout is [128 partitions = t%128, block-major
free dim], pre-swizzled on the host so every DMA is contiguous; small leading
input units and trailing solo output stores keep pipeline ramp/drain short.
"""

import numpy as np
import ml_dtypes

import concourse.bass as bass
import concourse.bacc as bacc
import concourse.mybir as mybir
from concourse.bass_utils import run_bass_kernel_spmd
from concourse.tile import TileContext

# Problem constants (hardcoded per harness contract).
B = 4096
T = 4096
DT = 1.0
PROCESS_VARIANCE = 1e-05
MEASUREMENT_VARIANCE = 0.1
INIT_ERROR = 1.0

N_CORES = 8
NCOLS = (B * 2) // N_CORES  # 1024 scalar sequences per core
CHUNK = 512                 # matmul free dim (one fp32 PSUM bank)
NBLK = T // 128             # 32 output chunks
NSLOT = 5                   # weight matrices: W0, Wlo1, Whi1, WloS, WhiS

BF16 = mybir.dt.bfloat16
F8 = mybir.dt.float8e4
I8 = mybir.dt.int8
F32 = mybir.dt.float32
NPBF16 = ml_dtypes.bfloat16
NPF8 = ml_dtypes.float8_e4m3

# Output quantization scales, folded per weight slot; the host divides them
# back out after the gather.  Transient blocks 0-1 see |y| up to 4.0 (the
# filter starts at gain ~1), steady blocks are ~N(0, 0.32^2).
OSCALE_T = 48.0   # blocks 0, 1 (slots 0-2)
OSCALE_S = 104.0  # blocks 2..31 (slots 3-4)
# First-order noise-shaping feedback coefficient (min ||g*h|| monic h).
SHAPE_FB = 0.9091


def _precompute_lhsT():
    """Host-side Riccati + chunk weight matrices, float64 -> bf16.

    Returns [128, 5*128] bf16; slot s holds lhsT = W_s.T so that
    matmul(out, lhsT, z) computes out[t, n] = sum_k W_s[t, k] z[k, n].
    """
    F = np.array([[1.0, DT], [0.0, 1.0]], dtype=np.float64)
    I2 = np.eye(2, dtype=np.float64)
    P = INIT_ERROR * I2.copy()
    A = np.zeros((T, 2, 2), dtype=np.float64)
    k = np.zeros((T, 2), dtype=np.float64)
    for t in range(T):
        Pp = F @ P @ F.T + PROCESS_VARIANCE * I2
        s = Pp[0, 0] + MEASUREMENT_VARIANCE
        kt = Pp[:, 0] / s
        k[t] = kt
        KH = np.zeros((2, 2), dtype=np.float64)
        KH[:, 0] = kt
        P = (I2 - KH) @ Pp
        A[t] = (I2 - KH) @ F

    # Exact input->output operator over the first 256 steps.  Rc[:, j] is the
    # coefficient of measurement z_j in the current state; the initial state
    # is x_{-1} = [z_0, 0].
    W = np.zeros((256, 256), dtype=np.float64)
    Rc = np.zeros((2, 256), dtype=np.float64)
    Rc[0, 0] = 1.0
    for t in range(256):
        Rc = A[t] @ Rc
        Rc[:, t] += k[t]
        W[t] = Rc[0]

    # Steady-state impulse response g_d = [Ainf^d kinf][0].
    g = np.zeros(256, dtype=np.float64)
    vv = k[-1].copy()
    for d in range(256):
        g[d] = vv[0]
        vv = A[-1] @ vv
    m, kk = np.mgrid[0:128, 0:128]
    WloS = g[m + 128 - kk]
    WhiS = np.where(m >= kk, g[np.abs(m - kk)], 0.0)

    slots = [
        (W[0:128, 0:128], OSCALE_T),      # chunk 0 (exact, incl. init cond)
        (W[128:256, 0:128], OSCALE_T),    # chunk 1 lo (exact transient)
        (W[128:256, 128:256], OSCALE_T),  # chunk 1 hi
        (WloS, OSCALE_S),                 # chunks 2..31 lo (steady Toeplitz)
        (WhiS, OSCALE_S),                 # chunks 2..31 hi
    ]
    lhsT = np.zeros((128, NSLOT * 128), dtype=np.float64)
    for s, (Ws, sc) in enumerate(slots):
        lhsT[:, s * 128:(s + 1) * 128] = Ws.T * sc
    return np.ascontiguousarray(lhsT.astype(NPBF16))


def _build_nc():
    # Device layout (host pre-swizzled): z/v are [128 partitions, 32 blocks x
    # 1024 seqs]; partition p of block bi holds time step 128*bi + p.  Every
    # DMA is then fully contiguous per partition line.
    nc = bacc.Bacc()
    z = nc.dram_tensor("z", [128, NBLK * NCOLS], F8, kind="ExternalInput")
    u = nc.dram_tensor("u", [128, NSLOT * 128], BF16, kind="ExternalInput")
    v = nc.dram_tensor("v", [128, NBLK * NCOLS], I8, kind="ExternalOutput")

    nchunks = NCOLS // CHUNK

    # DMA granularity: a dma_start's descriptor generation (DIRECT2D)
    # occupies its issuing HWDGE ring ~0.6-0.7us, and the two rings (sync /
    # scalar) generate in parallel.  fp8 blocks are 128 KiB; small leading
    # units shorten pipeline ramp, then 512 KiB groups of 4, alternating
    # rings so consecutive units' descriptor gens overlap.
    GROUP = 4
    # Input units: solos, then pairs through block 19 (finer completion
    # granularity where the stream runs just-in-time against the PE), then
    # quads.  The PE consumes a block per ~0.86us; input completions land
    # every ~0.7us per pair, so group-boundary stalls stay under the noise.
    in_units = [[0], [1]]
    bi = 2
    while bi < 20:
        in_units.append([bi, bi + 1])
        bi += 2
    while bi < NBLK:
        in_units.append(list(range(bi, bi + GROUP)))
        bi += GROUP
    # Output units: quads through block 23, then pairs, then solos — the
    # tail stores shrink so the post-stream drain is one small transfer.
    out_units = []
    ci = 0
    while ci < 24:
        out_units.append(list(range(ci, ci + GROUP)))
        ci += GROUP
    out_units += [[24, 25], [26, 27], [28], [29], [30], [31]]
    SOLO_OUT = 4
    out_unit_of = {}
    for unit in out_units:
        for c in unit:
            out_unit_of[c] = unit

    with TileContext(nc) as tc:
        with (
            tc.tile_pool(name="consts", bufs=1) as cpool,
            tc.tile_pool(name="zsolo", bufs=3) as zsolo,
            tc.tile_pool(name="zgrp", bufs=8) as zgrp,
            tc.tile_pool(name="vsolo", bufs=SOLO_OUT) as vsolo,
            tc.tile_pool(name="vgrp", bufs=6) as vgrp,
            tc.tile_pool(name="psum", bufs=8, space="PSUM") as ppool,
        ):
            # The SDMA engines starve the scalar ring's queue while the sync
            # ring has descriptors pending (observed ~2.5-3us stalls whenever
            # a block-0/1-critical transfer rode scalar).  So ALL inputs ride
            # the sync ring in consumption order; stores ride scalar.
            # Slot-0 weights first (they gate every matmul), then z block 0,
            # then the rest of the weights (needed from block 1 on).
            u_tile = cpool.tile([128, NSLOT * 128], BF16)
            nc.sync.dma_start(u_tile[:, 0:128], u[:, bass.ds(0, 128)])

            # PE HAM warm-up: the clock gate starts at 1.2 GHz and needs
            # ~3.4us of sustained activity to reach 2.4 GHz.  Burn scratch
            # matmuls (results never read) while the first input DMAs are in
            # flight.  The zero tile is memset on the otherwise-idle GpSimd
            # and serves as BOTH operands (fp8 stationary x fp8 moving), so
            # the warm-ups wait on nothing else — in particular not on the
            # weights DMA.  Two cold (1.2 GHz) warm-ups fill the ~0.9us gap
            # until z block 0 + slot-0 weights land.
            warm = zsolo.tile([128, 128 + CHUNK], F8, name="warm", tag="zp")
            nc.gpsimd.memset(warm[:, :], 0)
            for wi in range(2):
                wps = ppool.tile([128, CHUNK], F32, name=f"wps{wi}", tag="ps")
                nc.tensor.matmul(
                    wps[:, :],
                    warm[:, bass.ds(0, 128)],
                    warm[:, bass.ds(128, CHUNK)],
                    start=True,
                    stop=True,
                )

            # Input loads: fp8, units alternating between the two HWDGE
            # rings. zloc[bi]=(tile,col0).
            zloc = {}
            for ui, unit in enumerate(in_units):
                w = len(unit) * NCOLS
                pool = zsolo if len(unit) == 1 else zgrp
                zp = pool.tile([128, w], F8, name=f"z{unit[0]}", tag="zp")
                ring = nc.sync
                if unit == [0]:
                    # block 0 in two half-DMAs: the first matmul only needs
                    # cols 0:512, and tile deps are range-tracked, so the
                    # pipeline starts half a transfer earlier; the remaining
                    # weight slots follow, ahead of z block 1, matching
                    # consumption order
                    ring.dma_start(zp[:, 0:CHUNK], z[:, bass.ds(0, CHUNK)])
                    ring.dma_start(
                        zp[:, CHUNK:NCOLS], z[:, bass.ds(CHUNK, NCOLS - CHUNK)]
                    )
                    nc.sync.dma_start(
                        u_tile[:, 128:NSLOT * 128],
                        u[:, bass.ds(128, (NSLOT - 1) * 128)],
                    )
                else:
                    ring.dma_start(
                        zp[:, :], z[:, bass.ds(unit[0] * NCOLS, w)]
                    )
                for si, b in enumerate(unit):
                    zloc[b] = (zp, si * NCOLS)

            vloc = {}
            evict = 0
            for ci in range(NBLK):
                unit = out_unit_of[ci]
                if ci == unit[0]:
                    w = len(unit) * NCOLS
                    pool = vsolo if len(unit) == 1 else vgrp
                    vt = pool.tile([128, w], I8, name=f"v{ci}", tag="vout")
                    for si, c in enumerate(unit):
                        vloc[c] = (vt, si * NCOLS)
                vout, vcol0 = vloc[ci]
                zhi, hcol0 = zloc[ci]
                ps = [
                    ppool.tile([128, CHUNK], F32, name=f"ps{cc}", tag="ps")
                    for cc in range(nchunks)
                ]
                if ci == 0:
                    for cc in range(nchunks):
                        nc.tensor.matmul(
                            ps[cc][:, :],
                            u_tile[:, bass.ds(0, 128)],
                            zhi[:, bass.ds(hcol0 + cc * CHUNK, CHUNK)],
                            start=True,
                            stop=True,
                        )
                else:
                    zlo, lcol0 = zloc[ci - 1]
                    lo_slot, hi_slot = (1, 2) if ci == 1 else (3, 4)
                    # lo over both col-chunks, then hi: consecutive matmuls
                    # share the stationary operand.
                    for cc in range(nchunks):
                        nc.tensor.matmul(
                            ps[cc][:, :],
                            u_tile[:, bass.ds(lo_slot * 128, 128)],
                            zlo[:, bass.ds(lcol0 + cc * CHUNK, CHUNK)],
                            start=True,
                            stop=False,
                        )
                    for cc in range(nchunks):
                        nc.tensor.matmul(
                            ps[cc][:, :],
                            u_tile[:, bass.ds(hi_slot * 128, 128)],
                            zhi[:, bass.ds(hcol0 + cc * CHUNK, CHUNK)],
                            start=False,
                            stop=True,
                        )
                # split PSUM evictions across DVE and ACT, strictly
                # alternating so neither engine queues a long run
                if ci == NBLK - 1:
                    # final block: chunk 0 (psum ready one matmul early) goes
                    # whole on DVE; chunk 1 — the last psum — splits across
                    # BOTH engines so the final eviction finishes ~0.35us
                    # after the last matmul
                    nc.vector.tensor_copy(
                        vout[:, bass.ds(vcol0, CHUNK)], ps[0][:, :]
                    )
                    half = CHUNK // 2
                    nc.scalar.copy(
                        vout[:, bass.ds(vcol0 + CHUNK, half)], ps[1][:, 0:half]
                    )
                    nc.vector.tensor_copy(
                        vout[:, bass.ds(vcol0 + CHUNK + half, half)],
                        ps[1][:, half:CHUNK],
                    )
                else:
                    for cc in range(nchunks):
                        cols = bass.ds(vcol0 + cc * CHUNK, CHUNK)
                        if evict % 2 == 0:
                            nc.vector.tensor_copy(vout[:, cols], ps[cc][:, :])
                        else:
                            nc.scalar.copy(vout[:, cols], ps[cc][:, :])
                        evict += 1
                # Output stores alternate between the two HWDGE rings (sync /
                # scalar) so consecutive stores overlap with the input loads
                # that lead the sync ring.
                if ci == unit[-1]:
                    w = len(unit) * NCOLS
                    vt0, _ = vloc[unit[0]]
                    if ci == NBLK - 1:
                        # final block: halves on both rings so the tail
                        # descriptor gens and transfers run in parallel
                        half = w // 2
                        nc.sync.dma_start(
                            v[:, bass.ds(unit[0] * NCOLS, half)], vt0[:, 0:half]
                        )
                        nc.scalar.dma_start(
                            v[:, bass.ds(unit[0] * NCOLS + half, half)],
                            vt0[:, half:w],
                        )
                    elif unit[0] >= 24:
                        # trailing pairs/solos alternate rings — by now the
                        # sync ring's input queue has drained, so both get
                        # service
                        ring = nc.sync if unit[0] in (24, 28, 30) else nc.scalar
                        ring.dma_start(
                            v[:, bass.ds(unit[0] * NCOLS, w)], vt0[:, :]
                        )
                    else:
                        # bulk stores ride the (starvable) scalar ring; their
                        # only deadline is SBUF buffer recycling, which has
                        # ~24 blocks of slack
                        nc.scalar.dma_start(
                            v[:, bass.ds(unit[0] * NCOLS, w)], vt0[:, :]
                        )
    nc.finalize()  # Bacc.compile(): splits multi-waits, allocates registers
    return nc


_CACHE = {}


def _encode_fp8_shaped(zt: np.ndarray) -> np.ndarray:
    """First-order noise-shaped fp8e4m3 quantization along the time axis.

    zt: [T, ncols] float32.  Returns [T, ncols] fp8 codes whose rounding
    error is spectrally shaped by H(z) = 1 - SHAPE_FB * z^-1.
    """
    out = np.empty(zt.shape, dtype=NPF8)
    e = np.zeros(zt.shape[1], dtype=np.float32)
    for t in range(T):
        vrow = zt[t] + SHAPE_FB * e
        qrow = vrow.astype(NPF8)
        out[t] = qrow
        e = vrow - qrow.astype(np.float32)
    return out


def _run(x_seq: np.ndarray, trace: bool = False):
    if "nc" not in _CACHE:
        _CACHE["nc"] = _build_nc()
        _CACHE["u"] = _precompute_lhsT()
    nc = _CACHE["nc"]
    u_all = _CACHE["u"]

    x = np.asarray(x_seq, dtype=np.float32)
    assert x.shape == (B, T, 2), x.shape

    # [B, T, 2] -> [T, B*2]; column n = 2*b + c.  Noise-shaped fp8 encode,
    # then each core's [T, NCOLS] shard is swizzled into the device layout
    # [128, NBLK*NCOLS] (partition = t % 128, block-major free dim) so DMAs
    # are contiguous.
    zt = np.ascontiguousarray(x.transpose(1, 0, 2).reshape(T, B * 2))
    z8 = _encode_fp8_shaped(zt)

    def swizzle(a, ncols_lo, ncols_hi):
        nb = a.shape[0] // 128
        return np.ascontiguousarray(
            a[:, ncols_lo:ncols_hi]
            .reshape(nb, 128, ncols_hi - ncols_lo)
            .transpose(1, 0, 2)
            .reshape(128, nb * (ncols_hi - ncols_lo))
        )

    in_maps = [
        {"z": swizzle(z8, i * NCOLS, (i + 1) * NCOLS), "u": u_all}
        for i in range(N_CORES)
    ]
    res = run_bass_kernel_spmd(nc, in_maps, core_ids=list(range(N_CORES)), trace=trace)

    # inverse swizzle: [128, NBLK*NCOLS] -> [T, NCOLS], concat cores, dequant
    vt = np.concatenate(
        [
            r["v"].reshape(128, NBLK, NCOLS).transpose(1, 0, 2).reshape(T, NCOLS)
            for r in res.results
        ],
        axis=1,
    )  # [T, B*2] int8 = round(y * scale)
    vf = vt.astype(np.float32)
    vf[:256] /= OSCALE_T
    vf[256:] /= OSCALE_S
    out = np.ascontiguousarray(vf.reshape(T, B, 2).transpose(1, 0, 2))
    return out, res


def kernel(x_seq: np.ndarray) -> np.ndarray:
    out, _ = _run(x_seq, trace=False)
    return out


# revision 30
# speedup vs baseline: 1.0484x; 1.0259x over previous
"""Trainium2 Bass kernel for the batched 2D Kalman filter (nn_KalmanFilterWrapper).

Math
----
The reference runs, per trajectory, a Kalman filter over T=4096 steps with a
constant-velocity model.  The gain/covariance recursion (Riccati) is
data-independent, so the scan collapses to a linear time-varying recurrence

    x_t = A_t x_{t-1} + k_t z_t,        y_t = x_t[0]

with coefficients shared across the whole batch.  The 4-state filter decouples
into two identical 2-state (position, velocity) scalar filters — one per
coordinate — giving B*2 = 8192 independent scalar sequences.

The recurrence coefficients converge to steady state by t~135, and the steady
transition matrix has spectral radius 0.9315, so the filter's impulse response
g_d decays below 1e-6 by d=192.  Each aligned 128-step output chunk therefore
depends (to ~1e-5, vs a 2e-2 accuracy gate) only on the 256 measurements in
its own and the preceding 128-step input block:

    y[128*ci : 128*(ci+1)] = W_lo @ z_prev_block + W_hi @ z_this_block

where (W_lo, W_hi) are one shared Toeplitz pair built from g for all ci >= 2,
exact time-varying matrices for ci == 1, and a single exact lower-triangular
matrix for ci == 0 (which also folds in the x0 = [z_0, 0] initial condition).
All 32 chunks are INDEPENDENT matmuls — no serial carry chain at all.

Data movement (HBM floor ~358 GB/s per core; the kernel is PE-bound at
~27us of matmul streaming, with DMA at ~25us fully hidden under it):
  - inputs: fp8e4m3 with first-order NOISE-SHAPED quantization (error
    feedback e_t = v_t - fp8(v_t), v_t = z_t + 0.909 e_{t-1}) — 4 MB/core.
    The shaping pushes the fp8 rounding noise to high frequencies where the
    Kalman filter's transfer function is small: the in-band error is
    ||g*h||/||g|| = 0.42 of the raw 2.6e-2 fp8 error.  The fp8 tensor feeds
    the PE directly as the MOVING operand against bf16 stationary weights
    (mixed-dtype matmul, verified bit-exact on TRN2) — zero upconvert cost.
    ALL input transfers ride the sync HWDGE ring in exact consumption order
    (slot-0 weights, z block 0, remaining weights, z block 1, pairs, quads):
    the SDMA engines leave a freshly-doorbelled second queue unserviced for
    ~2-3us while the first has pending descriptors, so any early-needed
    transfer on the scalar ring stalls the PE (measured, repeatedly).
  - outputs: int8 round(y * s), saturating — 4 MB/core; s is folded into the
    weight slots (48 for the transient blocks 0-1 whose |y| reaches 4.0,
    104 for steady blocks where sigma(y)=0.32), and the host divides it back
    out after the gather.  PSUM evictions (fp32 -> int8, round-to-nearest)
    alternate between ACT and DVE at full copy speed.  Bulk stores ride the
    (starvable) scalar ring; the trailing pairs/solos alternate rings and
    the final block is split across both so the drain is short.
  - matmuls accumulate in fp32 PSUM.
Host-sim l2 relative error: 1.58e-2 (gate 2e-2, deterministic for the fixed
harness input); output int8 quantization ~0.95e-2 and shaped-fp8 input
~1.16e-2 dominate; bf16 weights + 256-tap truncation are ~0.2e-2.

Sharding: data-parallel across 8 NeuronCores, 512 trajectories (1024 scalar
sequences) per core.  Device layout is [128 partitions = t%128, block-major
free dim], pre-swizzled on the host so every DMA is contiguous; small leading
input units and trailing solo output stores keep pipeline ramp/drain short.
"""

import numpy as np
import ml_dtypes

import concourse.bass as bass
import concourse.bacc as bacc
import concourse.mybir as mybir
from concourse.bass_utils import run_bass_kernel_spmd
from concourse.tile import TileContext

# Problem constants (hardcoded per harness contract).
B = 4096
T = 4096
DT = 1.0
PROCESS_VARIANCE = 1e-05
MEASUREMENT_VARIANCE = 0.1
INIT_ERROR = 1.0

N_CORES = 8
NCOLS = (B * 2) // N_CORES  # 1024 scalar sequences per core
CHUNK = 512                 # matmul free dim (one fp32 PSUM bank)
NBLK = T // 128             # 32 output chunks
NSLOT = 5                   # weight matrices: W0, Wlo1, Whi1, WloS, WhiS

BF16 = mybir.dt.bfloat16
F8 = mybir.dt.float8e4
I8 = mybir.dt.int8
F32 = mybir.dt.float32
NPBF16 = ml_dtypes.bfloat16
NPF8 = ml_dtypes.float8_e4m3

# Output quantization scales, folded per weight slot; the host divides them
# back out after the gather.  Transient blocks 0-1 see |y| up to 4.0 (the
# filter starts at gain ~1), steady blocks are ~N(0, 0.32^2).
OSCALE_T = 48.0   # blocks 0, 1 (slots 0-2)
OSCALE_S = 104.0  # blocks 2..31 (slots 3-4)
# First-order noise-shaping feedback coefficient (min ||g*h|| monic h).
SHAPE_FB = 0.9091


def _precompute_lhsT():
    """Host-side Riccati + chunk weight matrices, float64 -> bf16.

    Returns [128, 5*128] bf16; slot s holds lhsT = W_s.T so that
    matmul(out, lhsT, z) computes out[t, n] = sum_k W_s[t, k] z[k, n].
    """
    F = np.array([[1.0, DT], [0.0, 1.0]], dtype=np.float64)
    I2 = np.eye(2, dtype=np.float64)
    P = INIT_ERROR * I2.copy()
    A = np.zeros((T, 2, 2), dtype=np.float64)
    k = np.zeros((T, 2), dtype=np.float64)
    for t in range(T):
        Pp = F @ P @ F.T + PROCESS_VARIANCE * I2
        s = Pp[0, 0] + MEASUREMENT_VARIANCE
        kt = Pp[:, 0] / s
        k[t] = kt
        KH = np.zeros((2, 2), dtype=np.float64)
        KH[:, 0] = kt
        P = (I2 - KH) @ Pp
        A[t] = (I2 - KH) @ F

    # Exact input->output operator over the first 256 steps.  Rc[:, j] is the
    # coefficient of measurement z_j in the current state; the initial state
    # is x_{-1} = [z_0, 0].
    W = np.zeros((256, 256), dtype=np.float64)
    Rc = np.zeros((2, 256), dtype=np.float64)
    Rc[0, 0] = 1.0
    for t in range(256):
        Rc = A[t] @ Rc
        Rc[:, t] += k[t]
        W[t] = Rc[0]

    # Steady-state impulse response g_d = [Ainf^d kinf][0].
    g = np.zeros(256, dtype=np.float64)
    vv = k[-1].copy()
    for d in range(256):
        g[d] = vv[0]
        vv = A[-1] @ vv
    m, kk = np.mgrid[0:128, 0:128]
    WloS = g[m + 128 - kk]
    WhiS = np.where(m >= kk, g[np.abs(m - kk)], 0.0)

    slots = [
        (W[0:128, 0:128], OSCALE_T),      # chunk 0 (exact, incl. init cond)
        (W[128:256, 0:128], OSCALE_T),    # chunk 1 lo (exact transient)
        (W[128:256, 128:256], OSCALE_T),  # chunk 1 hi
        (WloS, OSCALE_S),                 # chunks 2..31 lo (steady Toeplitz)
        (WhiS, OSCALE_S),                 # chunks 2..31 hi
    ]
    lhsT = np.zeros((128, NSLOT * 128), dtype=np.float64)
    for s, (Ws, sc) in enumerate(slots):
        lhsT[:, s * 128:(s + 1) * 128] = Ws.T * sc
    return np.ascontiguousarray(lhsT.astype(NPBF16))


def _build_nc():
    # Device layout (host pre-swizzled): z/v are [128 partitions, 32 blocks x
    # 1024 seqs]; partition p of block bi holds time step 128*bi + p.  Every
    # DMA is then fully contiguous per partition line.
    nc = bacc.Bacc()
    z = nc.dram_tensor("z", [128, NBLK * NCOLS], F8, kind="ExternalInput")
    u = nc.dram_tensor("u", [128, NSLOT * 128], BF16, kind="ExternalInput")
    v = nc.dram_tensor("v", [128, NBLK * NCOLS], I8, kind="ExternalOutput")

    nchunks = NCOLS // CHUNK

    # DMA granularity: a dma_start's descriptor generation (DIRECT2D)
    # occupies its issuing HWDGE ring ~0.6-0.7us, and the two rings (sync /
    # scalar) generate in parallel.  fp8 blocks are 128 KiB; small leading
    # units shorten pipeline ramp, then 512 KiB groups of 4, alternating
    # rings so consecutive units' descriptor gens overlap.
    GROUP = 4
    # Input units: solos, then pairs through block 19 (finer completion
    # granularity where the stream runs just-in-time against the PE), then
    # quads.  The PE consumes a block per ~0.86us; input completions land
    # every ~0.7us per pair, so group-boundary stalls stay under the noise.
    in_units = [[0], [1]]
    bi = 2
    while bi < 20:
        in_units.append([bi, bi + 1])
        bi += 2
    while bi < NBLK:
        in_units.append(list(range(bi, bi + GROUP)))
        bi += GROUP
    # Output units: quads through block 23, then pairs, then solos — the
    # tail stores shrink so the post-stream drain is one small transfer.
    out_units = []
    ci = 0
    while ci < 24:
        out_units.append(list(range(ci, ci + GROUP)))
        ci += GROUP
    out_units += [[24, 25], [26, 27], [28], [29], [30], [31]]
    SOLO_OUT = 4
    out_unit_of = {}
    for unit in out_units:
        for c in unit:
            out_unit_of[c] = unit

    with TileContext(nc) as tc:
        with (
            tc.tile_pool(name="consts", bufs=1) as cpool,
            tc.tile_pool(name="zsolo", bufs=3) as zsolo,
            tc.tile_pool(name="zgrp", bufs=8) as zgrp,
            tc.tile_pool(name="vsolo", bufs=SOLO_OUT) as vsolo,
            tc.tile_pool(name="vgrp", bufs=6) as vgrp,
            tc.tile_pool(name="psum", bufs=8, space="PSUM") as ppool,
        ):
            # The SDMA engines starve the scalar ring's queue while the sync
            # ring has descriptors pending (observed ~2.5-3us stalls whenever
            # a block-0/1-critical transfer rode scalar).  So ALL inputs ride
            # the sync ring in consumption order; stores ride scalar.
            # Slot-0 weights first (they gate every matmul), then z block 0,
            # then the rest of the weights (needed from block 1 on).
            u_tile = cpool.tile([128, NSLOT * 128], BF16)
            nc.sync.dma_start(u_tile[:, 0:128], u[:, bass.ds(0, 128)])

            # PE HAM warm-up: the clock gate starts at 1.2 GHz and needs
            # ~3.4us of sustained activity to reach 2.4 GHz.  Burn scratch
            # matmuls (results never read) while the first input DMAs are in
            # flight.  The zero tile is memset on the otherwise-idle GpSimd
            # and serves as BOTH operands (fp8 stationary x fp8 moving), so
            # the warm-ups wait on nothing else — in particular not on the
            # weights DMA.  Two cold (1.2 GHz) warm-ups fill the ~0.9us gap
            # until z block 0 + slot-0 weights land.
            warm = zsolo.tile([128, 128 + CHUNK], F8, name="warm", tag="zp")
            nc.gpsimd.memset(warm[:, :], 0)
            for wi in range(5):
                wps = ppool.tile([128, CHUNK], F32, name=f"wps{wi}", tag="ps")
                nc.tensor.matmul(
                    wps[:, :],
                    warm[:, bass.ds(0, 128)],
                    warm[:, bass.ds(128, CHUNK)],
                    start=True,
                    stop=True,
                )

            # Input loads: fp8, units alternating between the two HWDGE
            # rings. zloc[bi]=(tile,col0).
            zloc = {}
            for ui, unit in enumerate(in_units):
                w = len(unit) * NCOLS
                pool = zsolo if len(unit) == 1 else zgrp
                zp = pool.tile([128, w], F8, name=f"z{unit[0]}", tag="zp")
                ring = nc.sync
                if unit == [0]:
                    # block 0 in two half-DMAs: the first matmul only needs
                    # cols 0:512, and tile deps are range-tracked, so the
                    # pipeline starts half a transfer earlier; the remaining
                    # weight slots follow, ahead of z block 1, matching
                    # consumption order
                    ring.dma_start(zp[:, 0:CHUNK], z[:, bass.ds(0, CHUNK)])
                    ring.dma_start(
                        zp[:, CHUNK:NCOLS], z[:, bass.ds(CHUNK, NCOLS - CHUNK)]
                    )
                    nc.sync.dma_start(
                        u_tile[:, 128:NSLOT * 128],
                        u[:, bass.ds(128, (NSLOT - 1) * 128)],
                    )
                else:
                    ring.dma_start(
                        zp[:, :], z[:, bass.ds(unit[0] * NCOLS, w)]
                    )
                for si, b in enumerate(unit):
                    zloc[b] = (zp, si * NCOLS)

            vloc = {}
            evict = 0
            for ci in range(NBLK):
                unit = out_unit_of[ci]
                if ci == unit[0]:
                    w = len(unit) * NCOLS
                    pool = vsolo if len(unit) == 1 else vgrp
                    vt = pool.tile([128, w], I8, name=f"v{ci}", tag="vout")
                    for si, c in enumerate(unit):
                        vloc[c] = (vt, si * NCOLS)
                vout, vcol0 = vloc[ci]
                zhi, hcol0 = zloc[ci]
                ps = [
                    ppool.tile([128, CHUNK], F32, name=f"ps{cc}", tag="ps")
                    for cc in range(nchunks)
                ]
                if ci == 0:
                    for cc in range(nchunks):
                        nc.tensor.matmul(
                            ps[cc][:, :],
                            u_tile[:, bass.ds(0, 128)],
                            zhi[:, bass.ds(hcol0 + cc * CHUNK, CHUNK)],
                            start=True,
                            stop=True,
                        )
                else:
                    zlo, lcol0 = zloc[ci - 1]
                    lo_slot, hi_slot = (1, 2) if ci == 1 else (3, 4)
                    # lo over both col-chunks, then hi: consecutive matmuls
                    # share the stationary operand.
                    for cc in range(nchunks):
                        nc.tensor.matmul(
                            ps[cc][:, :],
                            u_tile[:, bass.ds(lo_slot * 128, 128)],
                            zlo[:, bass.ds(lcol0 + cc * CHUNK, CHUNK)],
                            start=True,
                            stop=False,
                        )
                    for cc in range(nchunks):
                        nc.tensor.matmul(
                            ps[cc][:, :],
                            u_tile[:, bass.ds(hi_slot * 128, 128)],
                            zhi[:, bass.ds(hcol0 + cc * CHUNK, CHUNK)],
                            start=False,
                            stop=True,
                        )
                # split PSUM evictions across DVE and ACT, strictly
                # alternating so neither engine queues a long run
                if ci == NBLK - 1:
                    # final block: chunk 0 (psum ready one matmul early) goes
                    # whole on DVE; chunk 1 — the last psum — splits across
                    # BOTH engines so the final eviction finishes ~0.35us
                    # after the last matmul
                    nc.vector.tensor_copy(
                        vout[:, bass.ds(vcol0, CHUNK)], ps[0][:, :]
                    )
                    half = CHUNK // 2
                    nc.scalar.copy(
                        vout[:, bass.ds(vcol0 + CHUNK, half)], ps[1][:, 0:half]
                    )
                    nc.vector.tensor_copy(
                        vout[:, bass.ds(vcol0 + CHUNK + half, half)],
                        ps[1][:, half:CHUNK],
                    )
                else:
                    for cc in range(nchunks):
                        cols = bass.ds(vcol0 + cc * CHUNK, CHUNK)
                        if evict % 2 == 0:
                            nc.vector.tensor_copy(vout[:, cols], ps[cc][:, :])
                        else:
                            nc.scalar.copy(vout[:, cols], ps[cc][:, :])
                        evict += 1
                # Output stores alternate between the two HWDGE rings (sync /
                # scalar) so consecutive stores overlap with the input loads
                # that lead the sync ring.
                if ci == unit[-1]:
                    w = len(unit) * NCOLS
                    vt0, _ = vloc[unit[0]]
                    if ci == NBLK - 1:
                        # final block: halves on both rings so the tail
                        # descriptor gens and transfers run in parallel
                        half = w // 2
                        nc.sync.dma_start(
                            v[:, bass.ds(unit[0] * NCOLS, half)], vt0[:, 0:half]
                        )
                        nc.scalar.dma_start(
                            v[:, bass.ds(unit[0] * NCOLS + half, half)],
                            vt0[:, half:w],
                        )
                    elif unit[0] >= 24:
                        # trailing pairs/solos alternate rings — by now the
                        # sync ring's input queue has drained, so both get
                        # service
                        ring = nc.sync if unit[0] in (24, 28, 30) else nc.scalar
                        ring.dma_start(
                            v[:, bass.ds(unit[0] * NCOLS, w)], vt0[:, :]
                        )
                    else:
                        # bulk stores ride the (starvable) scalar ring; their
                        # only deadline is SBUF buffer recycling, which has
                        # ~24 blocks of slack
                        nc.scalar.dma_start(
                            v[:, bass.ds(unit[0] * NCOLS, w)], vt0[:, :]
                        )
    nc.finalize()  # Bacc.compile(): splits multi-waits, allocates registers
    return nc


_CACHE = {}


def _encode_fp8_shaped(zt: np.ndarray) -> np.ndarray:
    """First-order noise-shaped fp8e4m3 quantization along the time axis.

    zt: [T, ncols] float32.  Returns [T, ncols] fp8 codes whose rounding
    error is spectrally shaped by H(z) = 1 - SHAPE_FB * z^-1.
    """
    out = np.empty(zt.shape, dtype=NPF8)
    e = np.zeros(zt.shape[1], dtype=np.float32)
    for t in range(T):
        vrow = zt[t] + SHAPE_FB * e
        qrow = vrow.astype(NPF8)
        out[t] = qrow
        e = vrow - qrow.astype(np.float32)
    return out


def _run(x_seq: np.ndarray, trace: bool = False):
    if "nc" not in _CACHE:
        _CACHE["nc"] = _build_nc()
        _CACHE["u"] = _precompute_lhsT()
    nc = _CACHE["nc"]
    u_all = _CACHE["u"]

    x = np.asarray(x_seq, dtype=np.float32)
    assert x.shape == (B, T, 2), x.shape

    # [B, T, 2] -> [T, B*2]; column n = 2*b + c.  Noise-shaped fp8 encode,
    # then each core's [T, NCOLS] shard is swizzled into the device layout
    # [128, NBLK*NCOLS] (partition = t % 128, block-major free dim) so DMAs
    # are contiguous.
    zt = np.ascontiguousarray(x.transpose(1, 0, 2).reshape(T, B * 2))
    z8 = _encode_fp8_shaped(zt)

    def swizzle(a, ncols_lo, ncols_hi):
        nb = a.shape[0] // 128
        return np.ascontiguousarray(
            a[:, ncols_lo:ncols_hi]
            .reshape(nb, 128, ncols_hi - ncols_lo)
            .transpose(1, 0, 2)
            .reshape(128, nb * (ncols_hi - ncols_lo))
        )

    in_maps = [
        {"z": swizzle(z8, i * NCOLS, (i + 1) * NCOLS), "u": u_all}
        for i in range(N_CORES)
    ]
    res = run_bass_kernel_spmd(nc, in_maps, core_ids=list(range(N_CORES)), trace=trace)

    # inverse swizzle: [128, NBLK*NCOLS] -> [T, NCOLS], concat cores, dequant
    vt = np.concatenate(
        [
            r["v"].reshape(128, NBLK, NCOLS).transpose(1, 0, 2).reshape(T, NCOLS)
            for r in res.results
        ],
        axis=1,
    )  # [T, B*2] int8 = round(y * scale)
    vf = vt.astype(np.float32)
    vf[:256] /= OSCALE_T
    vf[256:] /= OSCALE_S
    out = np.ascontiguousarray(vf.reshape(T, B, 2).transpose(1, 0, 2))
    return out, res


def kernel(x_seq: np.ndarray) -> np.ndarray:
    out, _ = _run(x_seq, trace=False)
    return out
